# revision 20
# baseline (speedup 1.0000x reference)
"""MoE QKV parallel linear for Trainium2, 8 NeuronCores.

Problem: out[t] = x[t] @ W[id[t]].T with x [16384, 2048] f32,
W [4, 3072, 2048] f32, id sorted int32 (tokens pre-grouped by expert).

Sharding: data-parallel over tokens with expert-pure shards (tokens are
sorted by expert, so each core gets one expert's contiguous token span,
padded to a common t_max so the SPMD program is uniform).

Device kernel (v2, W-stationary): per core, out^T[3072, t_max] =
(x @ W[e].T)^T computed as 24 sweeps over 128-wide QKV row-packets.
Per sweep n: psum tiles [128 qkv, block] accumulate over 16 k-tiles with
the W tile [128k, 128qkv] as the PE stationary operand and resident x^T
[128k, block] as the moving operand. Tokens are split into blocks <= 512
(psum bank limit) and >= 256 (fp32r/issue efficiency). x and W are cast
to bf16 on the host (absmax rel err ~2.5e-3, fp32 PSUM accumulation);
bf16 also halves HBM traffic and enables the PE fast weight load.
W is host-packed per expert as wp[n*128+kk, ko*128+j] = W[e][n*128+j,
ko*128+kk] so each sweep's weights arrive in one contiguous 512KB DMA.
Host transposes out^T shards back and scatters into the full output.
"""

import numpy as np
import ml_dtypes

import concourse.bacc as bacc
import concourse.mybir as mybir
import concourse.tile as tile

NCORES = 8
HIDDEN = 2048
QKV_OUT = 3072
P = 128
KO = HIDDEN // P          # 16 contraction tiles
NPK = QKV_OUT // P        # 24 qkv row-packets
BF16 = ml_dtypes.bfloat16

_cache: dict = {}


def _blocks(t: int) -> list[int]:
    """Split t columns into pieces <=512, all >=256 when t allows."""
    if t <= 512:
        return [t]
    nfull, rem = divmod(t, 512)
    if rem == 0:
        return [512] * nfull
    if rem >= 256:
        return [512] * nfull + [rem]
    # borrow from the last full block so both pieces land in [256, 512]
    a = (rem + 512) // 2
    return [512] * (nfull - 1) + [a, rem + 512 - a]


def _build(mt: int):
    """Bass module for one core: outT[3072, mt*128] = (x @ W.T)^T."""
    nc = bacc.Bacc("TRN2", target_bir_lowering=False, debug=False)
    tmax = mt * P
    bf16 = mybir.dt.bfloat16
    f32 = mybir.dt.float32

    # xp: host-packed x, block-major k-inner: xp[p, (b, ko, c)] =
    # x[tok = start_b + c, ko*128 + p], so each block's 16 k-tiles are one
    # contiguous per-partition range (one large DMA per block group).
    xp = nc.dram_tensor("xp", [P, KO * tmax], bf16, kind="ExternalInput")
    wp = nc.dram_tensor("wp", [QKV_OUT, HIDDEN], bf16, kind="ExternalInput")
    outT = nc.dram_tensor("outT", [QKV_OUT, tmax], f32, kind="ExternalOutput")

    blks = _blocks(tmax)
    starts = np.concatenate([[0], np.cumsum(blks)]).astype(int)
    nb = len(blks)
    # out DMA split point (flush first half of each sweep early)
    hb = max(1, min(nb - 1, 2))
    h0 = int(starts[hb])

    with tile.TileContext(nc) as tc:
        with (
            tc.tile_pool(name="xa", bufs=1) as xa,
            tc.tile_pool(name="wq", bufs=10) as wq,
            tc.tile_pool(name="pp", bufs=8, space="PSUM") as pp,
            tc.tile_pool(name="op", bufs=9) as op,
        ):
            # resident packed x: one tile. Block b0 arrives in 4-ko chunks
            # so the PE starts ~2us in; later blocks stream while a prologue
            # of b0-only columns keeps the PE fed until all of x lands.
            xt = xa.tile([P, KO * tmax], bf16, name="x", tag="x")
            b0w = int(starts[1])
            for k0, k1 in ((0, 1), (1, 2), (2, 4), (4, 8), (8, 16)):
                nc.sync.dma_start(out=xt[:, k0 * b0w:k1 * b0w],
                                  in_=xp[:, k0 * b0w:k1 * b0w])
            xpend = list(range(1, nb))  # later blocks paced via load_xb

            def load_xb():
                """Issue the next x block from the scalar stream, behind a
                compute-dependent copy, so it trails compute progress
                instead of racing the W stream for early HBM bandwidth."""
                if xpend:
                    b = xpend.pop(0)
                    c0, c1 = int(starts[b]) * KO, int(starts[b + 1]) * KO
                    nc.scalar.dma_start(out=xt[:, c0:c1], in_=xp[:, c0:c1])

            def x_slice(ko, b):
                c0 = int(starts[b])
                bw = int(starts[b + 1]) - c0
                return xt[:, c0 * KO + ko * bw: c0 * KO + (ko + 1) * bw]

            def load_w(n):
                w = wq.tile([P, HIDDEN], bf16, name=f"w_{n}", tag="w")
                nc.scalar.dma_start(out=w[:],
                                    in_=wp[n * P:(n + 1) * P, :])
                return w

            ots = {}

            def cell(n, b, w, copy_eng=None):
                """One (qkv-packet, token-block) accumulation + drain."""
                c0, c1 = int(starts[b]), int(starts[b + 1])
                ps = pp.tile([P, c1 - c0], f32, name=f"ps_{n}_{b}", tag="ps")
                for ko in range(KO):
                    nc.tensor.matmul(
                        ps[:], w[:, ko * P:(ko + 1) * P], x_slice(ko, b),
                        start=(ko == 0), stop=(ko == KO - 1),
                    )
                ot = ots[n]
                last = n == NPK - 1
                if copy_eng is None:
                    nc.vector.tensor_copy(ot[:, c0:c1], ps[:])
                else:
                    copy_eng.copy(ot[:, c0:c1], ps[:])
                if b == hb - 1 and not last:
                    nc.sync.dma_start(
                        out=outT[n * P:(n + 1) * P, :h0], in_=ot[:, :h0])
                if b == nb - 1 and not last:
                    nc.sync.dma_start(
                        out=outT[n * P:(n + 1) * P, h0:], in_=ot[:, h0:])
                if last:  # flush per block so the drain overlaps compute
                    nc.sync.dma_start(
                        out=outT[n * P:(n + 1) * P, c0:c1], in_=ot[:, c0:c1])

            PRO = min(6, NPK) if nb > 1 else 0
            wqd = {}
            if PRO >= 2:
                # W0/W1 in interleaved quarters: the pair-interleaved first
                # two columns need both at matching ko cadence.
                w0 = wq.tile([P, HIDDEN], bf16, name="w_0", tag="w")
                w1 = wq.tile([P, HIDDEN], bf16, name="w_1", tag="w")
                q = HIDDEN // 4
                for i in range(4):
                    nc.scalar.dma_start(out=w0[:, i * q:(i + 1) * q],
                                        in_=wp[0:P, i * q:(i + 1) * q])
                    nc.scalar.dma_start(out=w1[:, i * q:(i + 1) * q],
                                        in_=wp[P:2 * P, i * q:(i + 1) * q])
                wqd = {0: w0, 1: w1}
            for n in range(len(wqd), min(3, NPK)):
                wqd[n] = load_w(n)

            def w_of(n):
                if n not in wqd:
                    wqd[n] = load_w(n)
                return wqd[n]

            if PRO >= 2:
                # first two b0 cells interleaved at ko granularity: halves
                # the early x/W demand rate so the PE never outruns HBM.
                for n in (0, 1):
                    ots[n] = op.tile([P, tmax], f32, name=f"o_{n}", tag="o")
                w_of(3), w_of(4)
                c1 = int(starts[1])
                psA = pp.tile([P, c1], f32, name="ps_0_0", tag="ps")
                psB = pp.tile([P, c1], f32, name="ps_1_0", tag="ps")
                for ko in range(KO):
                    for ps, w in ((psA, w0), (psB, w1)):
                        nc.tensor.matmul(
                            ps[:], w[:, ko * P:(ko + 1) * P], x_slice(ko, 0),
                            start=(ko == 0), stop=(ko == KO - 1),
                        )
                nc.scalar.copy(ots[0][:, :c1], psA[:])
                nc.scalar.copy(ots[1][:, :c1], psB[:])
                load_xb()
            for n in range(2 if PRO >= 2 else 0, PRO):
                ots[n] = op.tile([P, tmax], f32, name=f"o_{n}", tag="o")
                if n + 3 < NPK:
                    w_of(n + 3)
                cell(n, 0, w_of(n), copy_eng=nc.scalar)
                load_xb()
            while xpend:
                load_xb()
            for n in range(NPK):
                if n >= PRO:
                    ots[n] = op.tile([P, tmax], f32, name=f"o_{n}", tag="o")
                    if n + 3 < NPK:
                        w_of(n + 3)
                w = wqd.pop(n)
                for b in range(0 if n >= PRO else 1, nb):
                    cell(n, b, w)
    nc.compile()
    return nc


def _plan(counts):
    """Allocate 8 cores to experts proportionally (largest remainder),
    then split each expert's token range into per-core contiguous spans.
    Returns (spans, t_max): spans[c] = (expert, start, length)."""
    total = int(counts.sum())
    ne = len(counts)
    active = [e for e in range(ne) if counts[e] > 0]
    quota = {e: counts[e] * NCORES / total for e in active}
    alloc = {e: max(1, int(quota[e])) for e in active}
    while sum(alloc.values()) > NCORES:  # too many mins; shrink largest
        shrinkable = [e for e in active if alloc[e] > 1]
        e = max(shrinkable, key=lambda e: alloc[e] - quota[e])
        alloc[e] -= 1
    rema = sorted(active, key=lambda e: quota[e] - alloc[e], reverse=True)
    i = 0
    while sum(alloc.values()) < NCORES:
        alloc[rema[i % len(rema)]] += 1
        i += 1
    spans = []
    starts = np.concatenate([[0], np.cumsum(counts)])
    for e in active:
        k = alloc[e]
        base, extra = divmod(int(counts[e]), k)
        off = int(starts[e])
        for j in range(k):
            ln = base + (1 if j < extra else 0)
            spans.append((e, off, ln))
            off += ln
    t_max = max(ln for _, _, ln in spans)
    t_max = max(P, -(-t_max // P) * P)
    return spans, t_max


def _runner(mt: int):
    """Compiled 8-core executor for the mt-tile module, cached so repeat
    kernel() calls skip jax retracing. Mirrors bass2jax.run_bass_via_pjrt's
    multi-core path (concat per-core inputs on axis 0 + shard_map)."""
    import jax
    from jax.sharding import Mesh, PartitionSpec
    from jax.experimental.shard_map import shard_map
    from concourse import bass2jax, mybir as mb

    nc = _build(mt)
    bass2jax.install_neuronx_cc_hook()

    part_name = nc.partition_id_tensor.name if nc.partition_id_tensor else None
    in_names, out_names, out_avals = [], [], []
    for alloc in nc.m.functions[0].allocations:
        if not isinstance(alloc, mb.MemoryLocationSet):
            continue
        name = alloc.memorylocations[0].name
        if alloc.kind == "ExternalInput":
            if name != part_name:
                in_names.append(name)
        elif alloc.kind == "ExternalOutput":
            out_names.append(name)
            out_avals.append(
                jax.core.ShapedArray(tuple(alloc.tensor_shape),
                                     mb.dt.np(alloc.dtype)))
    n_params = len(in_names)
    n_outs = len(out_names)
    bind_names = in_names + out_names + ([part_name] if part_name else [])

    def _body(*args):
        operands = list(args)
        if part_name:
            operands.append(bass2jax.partition_id_tensor())
        outs = bass2jax._bass_exec_p.bind(
            *operands,
            out_avals=tuple(out_avals),
            in_names=tuple(bind_names),
            out_names=tuple(out_names),
            lowering_input_output_aliases=(),
            sim_require_finite=True,
            sim_require_nnan=True,
            nc=nc,
        )
        return tuple(outs)

    devices = jax.devices()[:NCORES]
    mesh = Mesh(np.asarray(devices), ("core",))
    sharded = jax.jit(
        shard_map(_body, mesh=mesh,
                  in_specs=(PartitionSpec("core"),) * (n_params + n_outs),
                  out_specs=(PartitionSpec("core"),) * n_outs,
                  check_rep=False),
        donate_argnums=tuple(range(n_params, n_params + n_outs)),
        keep_unused=True,
    )

    def run(in_maps):
        concat_in = [
            np.concatenate([m[name] for m in in_maps], axis=0)
            for name in in_names
        ]
        zeros = [np.zeros((NCORES * a.shape[0], *a.shape[1:]), a.dtype)
                 for a in out_avals]
        outs = sharded(*concat_in, *zeros)
        return [
            {name: np.asarray(outs[i]).reshape(NCORES, *out_avals[i].shape)[c]
             for i, name in enumerate(out_names)}
            for c in range(NCORES)
        ]

    return run


def _pack_w(We: np.ndarray) -> np.ndarray:
    """wp[n*128+kk, ko*128+j] = We[n*128+j, ko*128+kk], cast bf16."""
    return np.ascontiguousarray(
        We.reshape(NPK, P, KO, P).transpose(0, 3, 2, 1)
        .reshape(QKV_OUT, HIDDEN).astype(BF16))


def _pack_x(xs: np.ndarray, blks) -> np.ndarray:
    """xp[p, (b, ko, c)] = xs[start_b + c, ko*128 + p] for block-major
    contiguous per-block DMA delivery. xs: [t_max, HIDDEN] bf16."""
    t_max = xs.shape[0]
    parts = []
    c0 = 0
    for bw in blks:
        chunk = xs[c0:c0 + bw].reshape(bw, KO, P).transpose(2, 1, 0)
        parts.append(chunk.reshape(P, KO * bw))
        c0 += bw
    return np.ascontiguousarray(np.concatenate(parts, axis=1))


def make_inputs(x, W, spans, t_max):
    blks = _blocks(t_max)
    wps = {}
    in_maps = []
    for e, off, ln in spans:
        if e not in wps:
            wps[e] = _pack_w(np.asarray(W[e]))
        xs = np.zeros((t_max, HIDDEN), dtype=BF16)
        xs[:ln] = x[off:off + ln].astype(BF16)
        in_maps.append({"xp": _pack_x(xs, blks), "wp": wps[e]})
    return in_maps


def kernel(x, W, modality_mapping):
    x = np.ascontiguousarray(np.asarray(x, dtype=np.float32))
    W = np.asarray(W, dtype=np.float32)
    mm = np.asarray(modality_mapping)

    perm = None
    if np.any(np.diff(mm) < 0):  # insurance: tokens not pre-sorted
        perm = np.argsort(mm, kind="stable")
        x = x[perm]
        mm = mm[perm]

    T = x.shape[0]
    E = W.shape[0]
    counts = np.bincount(mm.astype(np.int64), minlength=E)
    spans, t_max = _plan(counts)
    mt = t_max // P

    if mt not in _cache:
        _cache[mt] = _runner(mt)
    run = _cache[mt]

    results = run(make_inputs(x, W, spans, t_max))

    out = np.empty((T, QKV_OUT), dtype=np.float32)
    for c, (e, off, ln) in enumerate(spans):
        out[off:off + ln] = results[c]["outT"][:, :ln].T
    if perm is not None:
        inv = np.empty_like(perm)
        inv[perm] = np.arange(T)
        out = out[inv]
    return out


# revision 22
# speedup vs baseline: 1.0221x; 1.0221x over previous
"""MoE QKV parallel linear for Trainium2, 8 NeuronCores.

Problem: out[t] = x[t] @ W[id[t]].T with x [16384, 2048] f32,
W [4, 3072, 2048] f32, id sorted int32 (tokens pre-grouped by expert).

Sharding: data-parallel over tokens with expert-pure shards (tokens are
sorted by expert, so each core gets one expert's contiguous token span,
padded to a common t_max so the SPMD program is uniform).

Device kernel (v2, W-stationary): per core, out^T[3072, t_max] =
(x @ W[e].T)^T computed as 24 sweeps over 128-wide QKV row-packets.
Per sweep n: psum tiles [128 qkv, block] accumulate over 16 k-tiles with
the W tile [128k, 128qkv] as the PE stationary operand and resident x^T
[128k, block] as the moving operand. Tokens are split into blocks <= 512
(psum bank limit) and >= 256 (fp32r/issue efficiency). x and W are cast
to bf16 on the host (absmax rel err ~2.5e-3, fp32 PSUM accumulation);
bf16 also halves HBM traffic and enables the PE fast weight load.
W is host-packed per expert as wp[n*128+kk, ko*128+j] = W[e][n*128+j,
ko*128+kk] so each sweep's weights arrive in one contiguous 512KB DMA.
Host transposes out^T shards back and scatters into the full output.
"""

import numpy as np
import ml_dtypes

import concourse.bacc as bacc
import concourse.mybir as mybir
import concourse.tile as tile

NCORES = 8
HIDDEN = 2048
QKV_OUT = 3072
P = 128
KO = HIDDEN // P          # 16 contraction tiles
NPK = QKV_OUT // P        # 24 qkv row-packets
BF16 = ml_dtypes.bfloat16

_cache: dict = {}


def _blocks(t: int) -> list[int]:
    """Split t columns into pieces <=512, all >=256 when t allows."""
    if t <= 512:
        return [t]
    nfull, rem = divmod(t, 512)
    if rem == 0:
        return [512] * nfull
    if rem >= 256:
        return [512] * nfull + [rem]
    # borrow from the last full block so both pieces land in [256, 512]
    a = (rem + 512) // 2
    return [512] * (nfull - 1) + [a, rem + 512 - a]


def _build(mt: int):
    """Bass module for one core: outT[3072, mt*128] = (x @ W.T)^T."""
    nc = bacc.Bacc("TRN2", target_bir_lowering=False, debug=False)
    tmax = mt * P
    bf16 = mybir.dt.bfloat16
    f32 = mybir.dt.float32

    # xp: host-packed x, block-major k-inner: xp[p, (b, ko, c)] =
    # x[tok = start_b + c, ko*128 + p], so each block's 16 k-tiles are one
    # contiguous per-partition range (one large DMA per block group).
    xp = nc.dram_tensor("xp", [P, KO * tmax], bf16, kind="ExternalInput")
    wp = nc.dram_tensor("wp", [QKV_OUT, HIDDEN], bf16, kind="ExternalInput")
    outT = nc.dram_tensor("outT", [QKV_OUT, tmax], f32, kind="ExternalOutput")

    blks = _blocks(tmax)
    starts = np.concatenate([[0], np.cumsum(blks)]).astype(int)
    nb = len(blks)
    # out DMA split point (flush first half of each sweep early)
    hb = max(1, min(nb - 1, 2))
    h0 = int(starts[hb])

    with tile.TileContext(nc) as tc:
        with (
            tc.tile_pool(name="xa", bufs=1) as xa,
            tc.tile_pool(name="wq", bufs=11) as wq,
            tc.tile_pool(name="pp", bufs=8, space="PSUM") as pp,
            tc.tile_pool(name="op", bufs=10) as op,
        ):
            # resident packed x: one tile. Block b0 arrives in small ko
            # chunks so the PE starts ~10us in; later blocks stream while a
            # prologue of b0-only columns keeps the PE fed until x lands.
            xt = xa.tile([P, KO * tmax], bf16, name="x", tag="x")
            b0w = int(starts[1])
            for k0, k1 in ((0, 2), (2, 4), (4, 8), (8, 16)):
                nc.sync.dma_start(out=xt[:, k0 * b0w:k1 * b0w],
                                  in_=xp[:, k0 * b0w:k1 * b0w])
            for b in range(1, nb):
                c0, c1 = int(starts[b]) * KO, int(starts[b + 1]) * KO
                nc.sync.dma_start(out=xt[:, c0:c1], in_=xp[:, c0:c1])

            def x_slice(ko, b):
                c0 = int(starts[b])
                bw = int(starts[b + 1]) - c0
                return xt[:, c0 * KO + ko * bw: c0 * KO + (ko + 1) * bw]

            def load_w(n, split=False):
                w = wq.tile([P, HIDDEN], bf16, name=f"w_{n}", tag="w")
                if split:  # halves so the first matmuls unblock sooner
                    h = HIDDEN // 2
                    nc.scalar.dma_start(out=w[:, :h],
                                        in_=wp[n * P:(n + 1) * P, :h])
                    nc.scalar.dma_start(out=w[:, h:],
                                        in_=wp[n * P:(n + 1) * P, h:])
                else:
                    nc.scalar.dma_start(out=w[:],
                                        in_=wp[n * P:(n + 1) * P, :])
                return w

            ots = {}

            def cell(n, b, w):
                """One (qkv-packet, token-block) accumulation + drain."""
                c0, c1 = int(starts[b]), int(starts[b + 1])
                ps = pp.tile([P, c1 - c0], f32, name=f"ps_{n}_{b}", tag="ps")
                for ko in range(KO):
                    nc.tensor.matmul(
                        ps[:], w[:, ko * P:(ko + 1) * P], x_slice(ko, b),
                        start=(ko == 0), stop=(ko == KO - 1),
                    )
                ot = ots[n]
                nc.vector.tensor_copy(ot[:, c0:c1], ps[:])
                last = n == NPK - 1
                if b == hb - 1 and not last:
                    nc.sync.dma_start(
                        out=outT[n * P:(n + 1) * P, :h0], in_=ot[:, :h0])
                if b == nb - 1 and not last and h0 < tmax:
                    nc.sync.dma_start(
                        out=outT[n * P:(n + 1) * P, h0:], in_=ot[:, h0:])
                if last:  # per-block flush on the idle W ring: the drain
                    # overlaps compute and never queues behind sync traffic
                    nc.scalar.dma_start(
                        out=outT[n * P:(n + 1) * P, c0:c1], in_=ot[:, c0:c1])

            PRO = min(7, NPK) if nb > 1 else 0
            wqd = {n: load_w(n, split=(n == 0)) for n in range(min(3, NPK))}

            def w_of(n):
                if n not in wqd:
                    wqd[n] = load_w(n)
                return wqd[n]

            for n in range(PRO):  # prologue: b0-only columns
                ots[n] = op.tile([P, tmax], f32, name=f"o_{n}", tag="o")
                if n + 3 < NPK:
                    w_of(n + 3)
                cell(n, 0, w_of(n))
            for n in range(NPK):
                if n >= PRO:
                    ots[n] = op.tile([P, tmax], f32, name=f"o_{n}", tag="o")
                    if n + 3 < NPK:
                        w_of(n + 3)
                w = wqd.pop(n)
                for b in range(0 if n >= PRO else 1, nb):
                    cell(n, b, w)
    nc.compile()
    return nc


def _plan(counts):
    """Allocate 8 cores to experts proportionally (largest remainder),
    then split each expert's token range into per-core contiguous spans.
    Returns (spans, t_max): spans[c] = (expert, start, length)."""
    total = int(counts.sum())
    ne = len(counts)
    active = [e for e in range(ne) if counts[e] > 0]
    quota = {e: counts[e] * NCORES / total for e in active}
    alloc = {e: max(1, int(quota[e])) for e in active}
    while sum(alloc.values()) > NCORES:  # too many mins; shrink largest
        shrinkable = [e for e in active if alloc[e] > 1]
        e = max(shrinkable, key=lambda e: alloc[e] - quota[e])
        alloc[e] -= 1
    rema = sorted(active, key=lambda e: quota[e] - alloc[e], reverse=True)
    i = 0
    while sum(alloc.values()) < NCORES:
        alloc[rema[i % len(rema)]] += 1
        i += 1
    spans = []
    starts = np.concatenate([[0], np.cumsum(counts)])
    for e in active:
        k = alloc[e]
        base, extra = divmod(int(counts[e]), k)
        off = int(starts[e])
        for j in range(k):
            ln = base + (1 if j < extra else 0)
            spans.append((e, off, ln))
            off += ln
    t_max = max(ln for _, _, ln in spans)
    t_max = max(P, -(-t_max // P) * P)
    return spans, t_max


def _runner(mt: int):
    """Compiled 8-core executor for the mt-tile module, cached so repeat
    kernel() calls skip jax retracing. Mirrors bass2jax.run_bass_via_pjrt's
    multi-core path (concat per-core inputs on axis 0 + shard_map)."""
    import jax
    from jax.sharding import Mesh, PartitionSpec
    from jax.experimental.shard_map import shard_map
    from concourse import bass2jax, mybir as mb

    nc = _build(mt)
    bass2jax.install_neuronx_cc_hook()

    part_name = nc.partition_id_tensor.name if nc.partition_id_tensor else None
    in_names, out_names, out_avals = [], [], []
    for alloc in nc.m.functions[0].allocations:
        if not isinstance(alloc, mb.MemoryLocationSet):
            continue
        name = alloc.memorylocations[0].name
        if alloc.kind == "ExternalInput":
            if name != part_name:
                in_names.append(name)
        elif alloc.kind == "ExternalOutput":
            out_names.append(name)
            out_avals.append(
                jax.core.ShapedArray(tuple(alloc.tensor_shape),
                                     mb.dt.np(alloc.dtype)))
    n_params = len(in_names)
    n_outs = len(out_names)
    bind_names = in_names + out_names + ([part_name] if part_name else [])

    def _body(*args):
        operands = list(args)
        if part_name:
            operands.append(bass2jax.partition_id_tensor())
        outs = bass2jax._bass_exec_p.bind(
            *operands,
            out_avals=tuple(out_avals),
            in_names=tuple(bind_names),
            out_names=tuple(out_names),
            lowering_input_output_aliases=(),
            sim_require_finite=True,
            sim_require_nnan=True,
            nc=nc,
        )
        return tuple(outs)

    devices = jax.devices()[:NCORES]
    mesh = Mesh(np.asarray(devices), ("core",))
    sharded = jax.jit(
        shard_map(_body, mesh=mesh,
                  in_specs=(PartitionSpec("core"),) * (n_params + n_outs),
                  out_specs=(PartitionSpec("core"),) * n_outs,
                  check_rep=False),
        donate_argnums=tuple(range(n_params, n_params + n_outs)),
        keep_unused=True,
    )

    def run(in_maps):
        concat_in = [
            np.concatenate([m[name] for m in in_maps], axis=0)
            for name in in_names
        ]
        zeros = [np.zeros((NCORES * a.shape[0], *a.shape[1:]), a.dtype)
                 for a in out_avals]
        outs = sharded(*concat_in, *zeros)
        return [
            {name: np.asarray(outs[i]).reshape(NCORES, *out_avals[i].shape)[c]
             for i, name in enumerate(out_names)}
            for c in range(NCORES)
        ]

    return run


def _pack_w(We: np.ndarray) -> np.ndarray:
    """wp[n*128+kk, ko*128+j] = We[n*128+j, ko*128+kk], cast bf16."""
    return np.ascontiguousarray(
        We.reshape(NPK, P, KO, P).transpose(0, 3, 2, 1)
        .reshape(QKV_OUT, HIDDEN).astype(BF16))


def _pack_x(xs: np.ndarray, blks) -> np.ndarray:
    """xp[p, (b, ko, c)] = xs[start_b + c, ko*128 + p] for block-major
    contiguous per-block DMA delivery. xs: [t_max, HIDDEN] bf16."""
    t_max = xs.shape[0]
    parts = []
    c0 = 0
    for bw in blks:
        chunk = xs[c0:c0 + bw].reshape(bw, KO, P).transpose(2, 1, 0)
        parts.append(chunk.reshape(P, KO * bw))
        c0 += bw
    return np.ascontiguousarray(np.concatenate(parts, axis=1))


def make_inputs(x, W, spans, t_max):
    blks = _blocks(t_max)
    wps = {}
    in_maps = []
    for e, off, ln in spans:
        if e not in wps:
            wps[e] = _pack_w(np.asarray(W[e]))
        xs = np.zeros((t_max, HIDDEN), dtype=BF16)
        xs[:ln] = x[off:off + ln].astype(BF16)
        in_maps.append({"xp": _pack_x(xs, blks), "wp": wps[e]})
    return in_maps


def kernel(x, W, modality_mapping):
    x = np.ascontiguousarray(np.asarray(x, dtype=np.float32))
    W = np.asarray(W, dtype=np.float32)
    mm = np.asarray(modality_mapping)

    perm = None
    if np.any(np.diff(mm) < 0):  # insurance: tokens not pre-sorted
        perm = np.argsort(mm, kind="stable")
        x = x[perm]
        mm = mm[perm]

    T = x.shape[0]
    E = W.shape[0]
    counts = np.bincount(mm.astype(np.int64), minlength=E)
    spans, t_max = _plan(counts)
    mt = t_max // P

    if mt not in _cache:
        _cache[mt] = _runner(mt)
    run = _cache[mt]

    results = run(make_inputs(x, W, spans, t_max))

    out = np.empty((T, QKV_OUT), dtype=np.float32)
    for c, (e, off, ln) in enumerate(spans):
        out[off:off + ln] = results[c]["outT"][:, :ln].T
    if perm is not None:
        inv = np.empty_like(perm)
        inv[perm] = np.arange(T)
        out = out[inv]
    return out


# revision 23
# speedup vs baseline: 1.0287x; 1.0064x over previous
"""MoE QKV parallel linear for Trainium2, 8 NeuronCores.

Problem: out[t] = x[t] @ W[id[t]].T with x [16384, 2048] f32,
W [4, 3072, 2048] f32, id sorted int32 (tokens pre-grouped by expert).

Sharding: data-parallel over tokens with expert-pure shards (tokens are
sorted by expert, so each core gets one expert's contiguous token span,
padded to a common t_max so the SPMD program is uniform).

Device kernel (v2, W-stationary): per core, out^T[3072, t_max] =
(x @ W[e].T)^T computed as 24 sweeps over 128-wide QKV row-packets.
Per sweep n: psum tiles [128 qkv, block] accumulate over 16 k-tiles with
the W tile [128k, 128qkv] as the PE stationary operand and resident x^T
[128k, block] as the moving operand. Tokens are split into blocks <= 512
(psum bank limit) and >= 256 (fp32r/issue efficiency). x and W are cast
to bf16 on the host (absmax rel err ~2.5e-3, fp32 PSUM accumulation);
bf16 also halves HBM traffic and enables the PE fast weight load.
W is host-packed per expert as wp[n*128+kk, ko*128+j] = W[e][n*128+j,
ko*128+kk] so each sweep's weights arrive in one contiguous 512KB DMA.
Host transposes out^T shards back and scatters into the full output.
"""

import numpy as np
import ml_dtypes

import concourse.bacc as bacc
import concourse.mybir as mybir
import concourse.tile as tile

NCORES = 8
HIDDEN = 2048
QKV_OUT = 3072
P = 128
KO = HIDDEN // P          # 16 contraction tiles
NPK = QKV_OUT // P        # 24 qkv row-packets
BF16 = ml_dtypes.bfloat16

_cache: dict = {}


def _blocks(t: int) -> list[int]:
    """Split t columns into pieces <=512, all >=256 when t allows."""
    if t <= 512:
        return [t]
    nfull, rem = divmod(t, 512)
    if rem == 0:
        return [512] * nfull
    if rem >= 256:
        return [512] * nfull + [rem]
    # borrow from the last full block so both pieces land in [256, 512]
    a = (rem + 512) // 2
    return [512] * (nfull - 1) + [a, rem + 512 - a]


def _build(mt: int):
    """Bass module for one core: outT[3072, mt*128] = (x @ W.T)^T."""
    nc = bacc.Bacc("TRN2", target_bir_lowering=False, debug=False)
    tmax = mt * P
    bf16 = mybir.dt.bfloat16
    f32 = mybir.dt.float32

    # xp: host-packed x, block-major k-inner: xp[p, (b, ko, c)] =
    # x[tok = start_b + c, ko*128 + p], so each block's 16 k-tiles are one
    # contiguous per-partition range (one large DMA per block group).
    xp = nc.dram_tensor("xp", [P, KO * tmax], bf16, kind="ExternalInput")
    wp = nc.dram_tensor("wp", [QKV_OUT, HIDDEN], bf16, kind="ExternalInput")
    outT = nc.dram_tensor("outT", [QKV_OUT, tmax], f32, kind="ExternalOutput")

    blks = _blocks(tmax)
    starts = np.concatenate([[0], np.cumsum(blks)]).astype(int)
    nb = len(blks)
    # out DMA split point (flush first half of each sweep early)
    hb = max(1, min(nb - 1, 2))
    h0 = int(starts[hb])

    with tile.TileContext(nc) as tc:
        with (
            tc.tile_pool(name="xa", bufs=1) as xa,
            tc.tile_pool(name="wq", bufs=10) as wq,
            tc.tile_pool(name="pp", bufs=8, space="PSUM") as pp,
            tc.tile_pool(name="op", bufs=9) as op,
        ):
            # resident packed x: one tile. Block b0 arrives in small ko
            # chunks so the PE starts ~10us in; later blocks stream while a
            # prologue of b0-only columns keeps the PE fed until x lands.
            xt = xa.tile([P, KO * tmax], bf16, name="x", tag="x")
            b0w = int(starts[1])
            for k0, k1 in ((0, 4), (4, 8), (8, 12), (12, 16)):
                nc.sync.dma_start(out=xt[:, k0 * b0w:k1 * b0w],
                                  in_=xp[:, k0 * b0w:k1 * b0w])
            for b in range(1, nb):
                c0, c1 = int(starts[b]) * KO, int(starts[b + 1]) * KO
                nc.sync.dma_start(out=xt[:, c0:c1], in_=xp[:, c0:c1])

            def x_slice(ko, b):
                c0 = int(starts[b])
                bw = int(starts[b + 1]) - c0
                return xt[:, c0 * KO + ko * bw: c0 * KO + (ko + 1) * bw]

            def load_w(n, split=False):
                w = wq.tile([P, HIDDEN], bf16, name=f"w_{n}", tag="w")
                if split:  # halves so the first matmuls unblock sooner
                    h = HIDDEN // 2
                    nc.scalar.dma_start(out=w[:, :h],
                                        in_=wp[n * P:(n + 1) * P, :h])
                    nc.scalar.dma_start(out=w[:, h:],
                                        in_=wp[n * P:(n + 1) * P, h:])
                else:
                    nc.scalar.dma_start(out=w[:],
                                        in_=wp[n * P:(n + 1) * P, :])
                return w

            ots = {}

            def cell(n, b, w):
                """One (qkv-packet, token-block) accumulation + drain."""
                c0, c1 = int(starts[b]), int(starts[b + 1])
                ps = pp.tile([P, c1 - c0], f32, name=f"ps_{n}_{b}", tag="ps")
                for ko in range(KO):
                    nc.tensor.matmul(
                        ps[:], w[:, ko * P:(ko + 1) * P], x_slice(ko, b),
                        start=(ko == 0), stop=(ko == KO - 1),
                    )
                ot = ots[n]
                nc.vector.tensor_copy(ot[:, c0:c1], ps[:])
                last = n == NPK - 1
                if b == hb - 1 and not last:
                    nc.sync.dma_start(
                        out=outT[n * P:(n + 1) * P, :h0], in_=ot[:, :h0])
                if b == nb - 1 and not last and h0 < tmax:
                    nc.sync.dma_start(
                        out=outT[n * P:(n + 1) * P, h0:], in_=ot[:, h0:])
                if last:  # per-block flush on the idle W ring: the drain
                    # overlaps compute and never queues behind sync traffic
                    nc.scalar.dma_start(
                        out=outT[n * P:(n + 1) * P, c0:c1], in_=ot[:, c0:c1])

            PRO = min(6, NPK) if nb > 1 else 0
            wqd = {n: load_w(n) for n in range(min(3, NPK))}

            def w_of(n):
                if n not in wqd:
                    wqd[n] = load_w(n)
                return wqd[n]

            for n in range(PRO):  # prologue: b0-only columns
                ots[n] = op.tile([P, tmax], f32, name=f"o_{n}", tag="o")
                if n + 3 < NPK:
                    w_of(n + 3)
                cell(n, 0, w_of(n))
            for n in range(NPK):
                if n >= PRO:
                    ots[n] = op.tile([P, tmax], f32, name=f"o_{n}", tag="o")
                    if n + 3 < NPK:
                        w_of(n + 3)
                w = wqd.pop(n)
                for b in range(0 if n >= PRO else 1, nb):
                    cell(n, b, w)
    nc.compile()
    return nc


def _plan(counts):
    """Allocate 8 cores to experts proportionally (largest remainder),
    then split each expert's token range into per-core contiguous spans.
    Returns (spans, t_max): spans[c] = (expert, start, length)."""
    total = int(counts.sum())
    ne = len(counts)
    active = [e for e in range(ne) if counts[e] > 0]
    quota = {e: counts[e] * NCORES / total for e in active}
    alloc = {e: max(1, int(quota[e])) for e in active}
    while sum(alloc.values()) > NCORES:  # too many mins; shrink largest
        shrinkable = [e for e in active if alloc[e] > 1]
        e = max(shrinkable, key=lambda e: alloc[e] - quota[e])
        alloc[e] -= 1
    rema = sorted(active, key=lambda e: quota[e] - alloc[e], reverse=True)
    i = 0
    while sum(alloc.values()) < NCORES:
        alloc[rema[i % len(rema)]] += 1
        i += 1
    spans = []
    starts = np.concatenate([[0], np.cumsum(counts)])
    for e in active:
        k = alloc[e]
        base, extra = divmod(int(counts[e]), k)
        off = int(starts[e])
        for j in range(k):
            ln = base + (1 if j < extra else 0)
            spans.append((e, off, ln))
            off += ln
    t_max = max(ln for _, _, ln in spans)
    t_max = max(P, -(-t_max // P) * P)
    return spans, t_max


def _runner(mt: int):
    """Compiled 8-core executor for the mt-tile module, cached so repeat
    kernel() calls skip jax retracing. Mirrors bass2jax.run_bass_via_pjrt's
    multi-core path (concat per-core inputs on axis 0 + shard_map)."""
    import jax
    from jax.sharding import Mesh, PartitionSpec
    from jax.experimental.shard_map import shard_map
    from concourse import bass2jax, mybir as mb

    nc = _build(mt)
    bass2jax.install_neuronx_cc_hook()

    part_name = nc.partition_id_tensor.name if nc.partition_id_tensor else None
    in_names, out_names, out_avals = [], [], []
    for alloc in nc.m.functions[0].allocations:
        if not isinstance(alloc, mb.MemoryLocationSet):
            continue
        name = alloc.memorylocations[0].name
        if alloc.kind == "ExternalInput":
            if name != part_name:
                in_names.append(name)
        elif alloc.kind == "ExternalOutput":
            out_names.append(name)
            out_avals.append(
                jax.core.ShapedArray(tuple(alloc.tensor_shape),
                                     mb.dt.np(alloc.dtype)))
    n_params = len(in_names)
    n_outs = len(out_names)
    bind_names = in_names + out_names + ([part_name] if part_name else [])

    def _body(*args):
        operands = list(args)
        if part_name:
            operands.append(bass2jax.partition_id_tensor())
        outs = bass2jax._bass_exec_p.bind(
            *operands,
            out_avals=tuple(out_avals),
            in_names=tuple(bind_names),
            out_names=tuple(out_names),
            lowering_input_output_aliases=(),
            sim_require_finite=True,
            sim_require_nnan=True,
            nc=nc,
        )
        return tuple(outs)

    devices = jax.devices()[:NCORES]
    mesh = Mesh(np.asarray(devices), ("core",))
    sharded = jax.jit(
        shard_map(_body, mesh=mesh,
                  in_specs=(PartitionSpec("core"),) * (n_params + n_outs),
                  out_specs=(PartitionSpec("core"),) * n_outs,
                  check_rep=False),
        donate_argnums=tuple(range(n_params, n_params + n_outs)),
        keep_unused=True,
    )

    def run(in_maps):
        concat_in = [
            np.concatenate([m[name] for m in in_maps], axis=0)
            for name in in_names
        ]
        zeros = [np.zeros((NCORES * a.shape[0], *a.shape[1:]), a.dtype)
                 for a in out_avals]
        outs = sharded(*concat_in, *zeros)
        return [
            {name: np.asarray(outs[i]).reshape(NCORES, *out_avals[i].shape)[c]
             for i, name in enumerate(out_names)}
            for c in range(NCORES)
        ]

    return run


def _pack_w(We: np.ndarray) -> np.ndarray:
    """wp[n*128+kk, ko*128+j] = We[n*128+j, ko*128+kk], cast bf16."""
    return np.ascontiguousarray(
        We.reshape(NPK, P, KO, P).transpose(0, 3, 2, 1)
        .reshape(QKV_OUT, HIDDEN).astype(BF16))


def _pack_x(xs: np.ndarray, blks) -> np.ndarray:
    """xp[p, (b, ko, c)] = xs[start_b + c, ko*128 + p] for block-major
    contiguous per-block DMA delivery. xs: [t_max, HIDDEN] bf16."""
    t_max = xs.shape[0]
    parts = []
    c0 = 0
    for bw in blks:
        chunk = xs[c0:c0 + bw].reshape(bw, KO, P).transpose(2, 1, 0)
        parts.append(chunk.reshape(P, KO * bw))
        c0 += bw
    return np.ascontiguousarray(np.concatenate(parts, axis=1))


def make_inputs(x, W, spans, t_max):
    blks = _blocks(t_max)
    wps = {}
    in_maps = []
    for e, off, ln in spans:
        if e not in wps:
            wps[e] = _pack_w(np.asarray(W[e]))
        xs = np.zeros((t_max, HIDDEN), dtype=BF16)
        xs[:ln] = x[off:off + ln].astype(BF16)
        in_maps.append({"xp": _pack_x(xs, blks), "wp": wps[e]})
    return in_maps


def kernel(x, W, modality_mapping):
    x = np.ascontiguousarray(np.asarray(x, dtype=np.float32))
    W = np.asarray(W, dtype=np.float32)
    mm = np.asarray(modality_mapping)

    perm = None
    if np.any(np.diff(mm) < 0):  # insurance: tokens not pre-sorted
        perm = np.argsort(mm, kind="stable")
        x = x[perm]
        mm = mm[perm]

    T = x.shape[0]
    E = W.shape[0]
    counts = np.bincount(mm.astype(np.int64), minlength=E)
    spans, t_max = _plan(counts)
    mt = t_max // P

    if mt not in _cache:
        _cache[mt] = _runner(mt)
    run = _cache[mt]

    results = run(make_inputs(x, W, spans, t_max))

    out = np.empty((T, QKV_OUT), dtype=np.float32)
    for c, (e, off, ln) in enumerate(spans):
        out[off:off + ln] = results[c]["outT"][:, :ln].T
    if perm is not None:
        inv = np.empty_like(perm)
        inv[perm] = np.arange(T)
        out = out[inv]
    return out


# revision 24
# speedup vs baseline: 1.0333x; 1.0045x over previous
"""MoE QKV parallel linear for Trainium2, 8 NeuronCores.

Problem: out[t] = x[t] @ W[id[t]].T with x [16384, 2048] f32,
W [4, 3072, 2048] f32, id sorted int32 (tokens pre-grouped by expert).

Sharding: data-parallel over tokens with expert-pure shards (tokens are
sorted by expert, so each core gets one expert's contiguous token span,
padded to a common t_max so the SPMD program is uniform).

Device kernel (v2, W-stationary): per core, out^T[3072, t_max] =
(x @ W[e].T)^T computed as 24 sweeps over 128-wide QKV row-packets.
Per sweep n: psum tiles [128 qkv, block] accumulate over 16 k-tiles with
the W tile [128k, 128qkv] as the PE stationary operand and resident x^T
[128k, block] as the moving operand. Tokens are split into blocks <= 512
(psum bank limit) and >= 256 (fp32r/issue efficiency). x and W are cast
to bf16 on the host (absmax rel err ~2.5e-3, fp32 PSUM accumulation);
bf16 also halves HBM traffic and enables the PE fast weight load.
W is host-packed per expert as wp[n*128+kk, ko*128+j] = W[e][n*128+j,
ko*128+kk] so each sweep's weights arrive in one contiguous 512KB DMA.
Host transposes out^T shards back and scatters into the full output.
"""

import numpy as np
import ml_dtypes

import concourse.bacc as bacc
import concourse.mybir as mybir
import concourse.tile as tile

NCORES = 8
HIDDEN = 2048
QKV_OUT = 3072
P = 128
KO = HIDDEN // P          # 16 contraction tiles
NPK = QKV_OUT // P        # 24 qkv row-packets
BF16 = ml_dtypes.bfloat16

_cache: dict = {}


def _blocks(t: int) -> list[int]:
    """Split t columns into pieces <=512, all >=256 when t allows."""
    if t <= 512:
        return [t]
    nfull, rem = divmod(t, 512)
    if rem == 0:
        return [512] * nfull
    if rem >= 256:
        return [512] * nfull + [rem]
    # borrow from the last full block so both pieces land in [256, 512]
    a = (rem + 512) // 2
    return [512] * (nfull - 1) + [a, rem + 512 - a]


def _build(mt: int):
    """Bass module for one core: outT[3072, mt*128] = (x @ W.T)^T."""
    nc = bacc.Bacc("TRN2", target_bir_lowering=False, debug=False)
    tmax = mt * P
    bf16 = mybir.dt.bfloat16
    f32 = mybir.dt.float32

    # xp: host-packed x, block-major k-inner: xp[p, (b, ko, c)] =
    # x[tok = start_b + c, ko*128 + p], so each block's 16 k-tiles are one
    # contiguous per-partition range (one large DMA per block group).
    xp = nc.dram_tensor("xp", [P, KO * tmax], bf16, kind="ExternalInput")
    wp = nc.dram_tensor("wp", [QKV_OUT, HIDDEN], bf16, kind="ExternalInput")
    outT = nc.dram_tensor("outT", [QKV_OUT, tmax], f32, kind="ExternalOutput")

    blks = _blocks(tmax)
    starts = np.concatenate([[0], np.cumsum(blks)]).astype(int)
    nb = len(blks)
    # out DMA split point (flush first half of each sweep early)
    hb = max(1, min(nb - 1, 2))
    h0 = int(starts[hb])

    with tile.TileContext(nc) as tc:
        with (
            tc.tile_pool(name="xa", bufs=1) as xa,
            tc.tile_pool(name="wq", bufs=10) as wq,
            tc.tile_pool(name="pp", bufs=8, space="PSUM") as pp,
            tc.tile_pool(name="op", bufs=9) as op,
        ):
            # resident packed x: one tile. Block b0 arrives in small ko
            # chunks so the PE starts ~10us in; later blocks stream while a
            # prologue of b0-only columns keeps the PE fed until x lands.
            xt = xa.tile([P, KO * tmax], bf16, name="x", tag="x")
            b0w = int(starts[1])
            for k0, k1 in ((0, 4), (4, 8), (8, 12), (12, 16)):
                nc.sync.dma_start(out=xt[:, k0 * b0w:k1 * b0w],
                                  in_=xp[:, k0 * b0w:k1 * b0w])
            for b in range(1, nb):
                c0, c1 = int(starts[b]) * KO, int(starts[b + 1]) * KO
                nc.sync.dma_start(out=xt[:, c0:c1], in_=xp[:, c0:c1])

            def x_slice(ko, b):
                c0 = int(starts[b])
                bw = int(starts[b + 1]) - c0
                return xt[:, c0 * KO + ko * bw: c0 * KO + (ko + 1) * bw]

            def load_w(n, split=False):
                w = wq.tile([P, HIDDEN], bf16, name=f"w_{n}", tag="w")
                if split:  # halves so the first matmuls unblock sooner
                    h = HIDDEN // 2
                    nc.scalar.dma_start(out=w[:, :h],
                                        in_=wp[n * P:(n + 1) * P, :h])
                    nc.scalar.dma_start(out=w[:, h:],
                                        in_=wp[n * P:(n + 1) * P, h:])
                else:
                    nc.scalar.dma_start(out=w[:],
                                        in_=wp[n * P:(n + 1) * P, :])
                return w

            ots = {}

            def cell(n, b, w):
                """One (qkv-packet, token-block) accumulation + drain."""
                c0, c1 = int(starts[b]), int(starts[b + 1])
                ps = pp.tile([P, c1 - c0], f32, name=f"ps_{n}_{b}", tag="ps")
                for ko in range(KO):
                    nc.tensor.matmul(
                        ps[:], w[:, ko * P:(ko + 1) * P], x_slice(ko, b),
                        start=(ko == 0), stop=(ko == KO - 1),
                    )
                ot = ots[n]
                nc.vector.tensor_copy(ot[:, c0:c1], ps[:])
                last = n == NPK - 1
                if b == hb - 1 and not last:
                    nc.sync.dma_start(
                        out=outT[n * P:(n + 1) * P, :h0], in_=ot[:, :h0])
                if b == nb - 1 and not last and h0 < tmax:
                    nc.sync.dma_start(
                        out=outT[n * P:(n + 1) * P, h0:], in_=ot[:, h0:])
                if last:  # per-block flush on the idle W ring: the drain
                    # overlaps compute and never queues behind sync traffic
                    nc.scalar.dma_start(
                        out=outT[n * P:(n + 1) * P, c0:c1], in_=ot[:, c0:c1])

            PRO = min(7, NPK) if nb > 1 else 0
            wqd = {n: load_w(n) for n in range(min(3, NPK))}

            def w_of(n):
                if n not in wqd:
                    wqd[n] = load_w(n)
                return wqd[n]

            for n in range(PRO):  # prologue: b0-only columns
                ots[n] = op.tile([P, tmax], f32, name=f"o_{n}", tag="o")
                if n + 3 < NPK:
                    w_of(n + 3)
                cell(n, 0, w_of(n))
            for n in range(NPK):
                if n >= PRO:
                    ots[n] = op.tile([P, tmax], f32, name=f"o_{n}", tag="o")
                    if n + 3 < NPK:
                        w_of(n + 3)
                w = wqd.pop(n)
                for b in range(0 if n >= PRO else 1, nb):
                    cell(n, b, w)
    nc.compile()
    return nc


def _plan(counts):
    """Allocate 8 cores to experts proportionally (largest remainder),
    then split each expert's token range into per-core contiguous spans.
    Returns (spans, t_max): spans[c] = (expert, start, length)."""
    total = int(counts.sum())
    ne = len(counts)
    active = [e for e in range(ne) if counts[e] > 0]
    quota = {e: counts[e] * NCORES / total for e in active}
    alloc = {e: max(1, int(quota[e])) for e in active}
    while sum(alloc.values()) > NCORES:  # too many mins; shrink largest
        shrinkable = [e for e in active if alloc[e] > 1]
        e = max(shrinkable, key=lambda e: alloc[e] - quota[e])
        alloc[e] -= 1
    rema = sorted(active, key=lambda e: quota[e] - alloc[e], reverse=True)
    i = 0
    while sum(alloc.values()) < NCORES:
        alloc[rema[i % len(rema)]] += 1
        i += 1
    spans = []
    starts = np.concatenate([[0], np.cumsum(counts)])
    for e in active:
        k = alloc[e]
        base, extra = divmod(int(counts[e]), k)
        off = int(starts[e])
        for j in range(k):
            ln = base + (1 if j < extra else 0)
            spans.append((e, off, ln))
            off += ln
    t_max = max(ln for _, _, ln in spans)
    t_max = max(P, -(-t_max // P) * P)
    return spans, t_max


def _runner(mt: int):
    """Compiled 8-core executor for the mt-tile module, cached so repeat
    kernel() calls skip jax retracing. Mirrors bass2jax.run_bass_via_pjrt's
    multi-core path (concat per-core inputs on axis 0 + shard_map)."""
    import jax
    from jax.sharding import Mesh, PartitionSpec
    from jax.experimental.shard_map import shard_map
    from concourse import bass2jax, mybir as mb

    nc = _build(mt)
    bass2jax.install_neuronx_cc_hook()

    part_name = nc.partition_id_tensor.name if nc.partition_id_tensor else None
    in_names, out_names, out_avals = [], [], []
    for alloc in nc.m.functions[0].allocations:
        if not isinstance(alloc, mb.MemoryLocationSet):
            continue
        name = alloc.memorylocations[0].name
        if alloc.kind == "ExternalInput":
            if name != part_name:
                in_names.append(name)
        elif alloc.kind == "ExternalOutput":
            out_names.append(name)
            out_avals.append(
                jax.core.ShapedArray(tuple(alloc.tensor_shape),
                                     mb.dt.np(alloc.dtype)))
    n_params = len(in_names)
    n_outs = len(out_names)
    bind_names = in_names + out_names + ([part_name] if part_name else [])

    def _body(*args):
        operands = list(args)
        if part_name:
            operands.append(bass2jax.partition_id_tensor())
        outs = bass2jax._bass_exec_p.bind(
            *operands,
            out_avals=tuple(out_avals),
            in_names=tuple(bind_names),
            out_names=tuple(out_names),
            lowering_input_output_aliases=(),
            sim_require_finite=True,
            sim_require_nnan=True,
            nc=nc,
        )
        return tuple(outs)

    devices = jax.devices()[:NCORES]
    mesh = Mesh(np.asarray(devices), ("core",))
    sharded = jax.jit(
        shard_map(_body, mesh=mesh,
                  in_specs=(PartitionSpec("core"),) * (n_params + n_outs),
                  out_specs=(PartitionSpec("core"),) * n_outs,
                  check_rep=False),
        donate_argnums=tuple(range(n_params, n_params + n_outs)),
        keep_unused=True,
    )

    def run(in_maps):
        concat_in = [
            np.concatenate([m[name] for m in in_maps], axis=0)
            for name in in_names
        ]
        zeros = [np.zeros((NCORES * a.shape[0], *a.shape[1:]), a.dtype)
                 for a in out_avals]
        outs = sharded(*concat_in, *zeros)
        return [
            {name: np.asarray(outs[i]).reshape(NCORES, *out_avals[i].shape)[c]
             for i, name in enumerate(out_names)}
            for c in range(NCORES)
        ]

    return run


def _pack_w(We: np.ndarray) -> np.ndarray:
    """wp[n*128+kk, ko*128+j] = We[n*128+j, ko*128+kk], cast bf16."""
    return np.ascontiguousarray(
        We.reshape(NPK, P, KO, P).transpose(0, 3, 2, 1)
        .reshape(QKV_OUT, HIDDEN).astype(BF16))


def _pack_x(xs: np.ndarray, blks) -> np.ndarray:
    """xp[p, (b, ko, c)] = xs[start_b + c, ko*128 + p] for block-major
    contiguous per-block DMA delivery. xs: [t_max, HIDDEN] bf16."""
    t_max = xs.shape[0]
    parts = []
    c0 = 0
    for bw in blks:
        chunk = xs[c0:c0 + bw].reshape(bw, KO, P).transpose(2, 1, 0)
        parts.append(chunk.reshape(P, KO * bw))
        c0 += bw
    return np.ascontiguousarray(np.concatenate(parts, axis=1))


def make_inputs(x, W, spans, t_max):
    blks = _blocks(t_max)
    wps = {}
    in_maps = []
    for e, off, ln in spans:
        if e not in wps:
            wps[e] = _pack_w(np.asarray(W[e]))
        xs = np.zeros((t_max, HIDDEN), dtype=BF16)
        xs[:ln] = x[off:off + ln].astype(BF16)
        in_maps.append({"xp": _pack_x(xs, blks), "wp": wps[e]})
    return in_maps


def kernel(x, W, modality_mapping):
    x = np.ascontiguousarray(np.asarray(x, dtype=np.float32))
    W = np.asarray(W, dtype=np.float32)
    mm = np.asarray(modality_mapping)

    perm = None
    if np.any(np.diff(mm) < 0):  # insurance: tokens not pre-sorted
        perm = np.argsort(mm, kind="stable")
        x = x[perm]
        mm = mm[perm]

    T = x.shape[0]
    E = W.shape[0]
    counts = np.bincount(mm.astype(np.int64), minlength=E)
    spans, t_max = _plan(counts)
    mt = t_max // P

    if mt not in _cache:
        _cache[mt] = _runner(mt)
    run = _cache[mt]

    results = run(make_inputs(x, W, spans, t_max))

    out = np.empty((T, QKV_OUT), dtype=np.float32)
    for c, (e, off, ln) in enumerate(spans):
        out[off:off + ln] = results[c]["outT"][:, :ln].T
    if perm is not None:
        inv = np.empty_like(perm)
        inv[perm] = np.arange(T)
        out = out[inv]
    return out


# revision 26
# speedup vs baseline: 1.0763x; 1.0415x over previous
"""MoE QKV parallel linear for Trainium2, 8 NeuronCores.

Problem: out[t] = x[t] @ W[id[t]].T with x [16384, 2048] f32,
W [4, 3072, 2048] f32, id sorted int32 (tokens pre-grouped by expert).

Sharding: data-parallel over tokens with expert-pure shards (tokens are
sorted by expert, so each core gets one expert's contiguous token span,
padded to a common t_max so the SPMD program is uniform).

Device kernel (v2, W-stationary): per core, out^T[3072, t_max] =
(x @ W[e].T)^T computed as 24 sweeps over 128-wide QKV row-packets.
Per sweep n: psum tiles [128 qkv, block] accumulate over 16 k-tiles with
the W tile [128k, 128qkv] as the PE stationary operand and resident x^T
[128k, block] as the moving operand. Tokens are split into blocks <= 512
(psum bank limit) and >= 256 (fp32r/issue efficiency). x and W are cast
to bf16 on the host (absmax rel err ~2.5e-3, fp32 PSUM accumulation);
bf16 also halves HBM traffic and enables the PE fast weight load.
W is host-packed per expert as wp[n*128+kk, ko*128+j] = W[e][n*128+j,
ko*128+kk] so each sweep's weights arrive in one contiguous 512KB DMA.
Host transposes out^T shards back and scatters into the full output.
"""

import numpy as np
import ml_dtypes

import concourse.bacc as bacc
import concourse.mybir as mybir
import concourse.tile as tile

NCORES = 8
HIDDEN = 2048
QKV_OUT = 3072
P = 128
KO = HIDDEN // P          # 16 contraction tiles
NPK = QKV_OUT // P        # 24 qkv row-packets
BF16 = ml_dtypes.bfloat16

_cache: dict = {}


def _blocks(t: int) -> list[int]:
    """Split t columns into pieces <=512, all >=256 when t allows."""
    if t <= 512:
        return [t]
    nfull, rem = divmod(t, 512)
    if rem == 0:
        return [512] * nfull
    if rem >= 256:
        return [512] * nfull + [rem]
    # borrow from the last full block so both pieces land in [256, 512]
    a = (rem + 512) // 2
    return [512] * (nfull - 1) + [a, rem + 512 - a]


def _build(tmax: int):
    """Bass module for one core: outT[3072, tmax] = (x @ W.T)^T."""
    nc = bacc.Bacc("TRN2", target_bir_lowering=False, debug=False)
    bf16 = mybir.dt.bfloat16
    f32 = mybir.dt.float32

    # xp: host-packed x, block-major k-inner: xp[p, (b, ko, c)] =
    # x[tok = start_b + c, ko*128 + p], so each block's 16 k-tiles are one
    # contiguous per-partition range (one large DMA per block group).
    xp = nc.dram_tensor("xp", [P, KO * tmax], bf16, kind="ExternalInput")
    wp = nc.dram_tensor("wp", [QKV_OUT, HIDDEN], bf16, kind="ExternalInput")
    outT = nc.dram_tensor("outT", [QKV_OUT, tmax], f32, kind="ExternalOutput")

    blks = _blocks(tmax)
    starts = np.concatenate([[0], np.cumsum(blks)]).astype(int)
    nb = len(blks)
    # out DMA split point (flush first half of each sweep early)
    hb = max(1, min(nb - 1, 2))
    h0 = int(starts[hb])

    with tile.TileContext(nc) as tc:
        with (
            tc.tile_pool(name="xa", bufs=1) as xa,
            tc.tile_pool(name="wq", bufs=10) as wq,
            tc.tile_pool(name="pp", bufs=8, space="PSUM") as pp,
            tc.tile_pool(name="op", bufs=9) as op,
        ):
            # resident packed x: one tile. Block b0 arrives in small ko
            # chunks so the PE starts ~10us in; later blocks stream while a
            # prologue of b0-only columns keeps the PE fed until x lands.
            xt = xa.tile([P, KO * tmax], bf16, name="x", tag="x")
            b0w = int(starts[1])
            for k0, k1 in ((0, 4), (4, 8), (8, 12), (12, 16)):
                nc.sync.dma_start(out=xt[:, k0 * b0w:k1 * b0w],
                                  in_=xp[:, k0 * b0w:k1 * b0w])
            for b in range(1, nb):
                c0, c1 = int(starts[b]) * KO, int(starts[b + 1]) * KO
                nc.sync.dma_start(out=xt[:, c0:c1], in_=xp[:, c0:c1])

            def x_slice(ko, b):
                c0 = int(starts[b])
                bw = int(starts[b + 1]) - c0
                return xt[:, c0 * KO + ko * bw: c0 * KO + (ko + 1) * bw]

            def load_w(n, split=False):
                w = wq.tile([P, HIDDEN], bf16, name=f"w_{n}", tag="w")
                if split:  # halves so the first matmuls unblock sooner
                    h = HIDDEN // 2
                    nc.scalar.dma_start(out=w[:, :h],
                                        in_=wp[n * P:(n + 1) * P, :h])
                    nc.scalar.dma_start(out=w[:, h:],
                                        in_=wp[n * P:(n + 1) * P, h:])
                else:
                    nc.scalar.dma_start(out=w[:],
                                        in_=wp[n * P:(n + 1) * P, :])
                return w

            ots = {}

            def cell(n, b, w):
                """One (qkv-packet, token-block) accumulation + drain."""
                c0, c1 = int(starts[b]), int(starts[b + 1])
                ps = pp.tile([P, c1 - c0], f32, name=f"ps_{n}_{b}", tag="ps")
                for ko in range(KO):
                    nc.tensor.matmul(
                        ps[:], w[:, ko * P:(ko + 1) * P], x_slice(ko, b),
                        start=(ko == 0), stop=(ko == KO - 1),
                    )
                ot = ots[n]
                nc.vector.tensor_copy(ot[:, c0:c1], ps[:])
                last = n == NPK - 1
                if b == hb - 1 and not last:
                    nc.sync.dma_start(
                        out=outT[n * P:(n + 1) * P, :h0], in_=ot[:, :h0])
                if b == nb - 1 and not last and h0 < tmax:
                    nc.sync.dma_start(
                        out=outT[n * P:(n + 1) * P, h0:], in_=ot[:, h0:])
                if last:  # per-block flush on the idle W ring: the drain
                    # overlaps compute and never queues behind sync traffic
                    nc.scalar.dma_start(
                        out=outT[n * P:(n + 1) * P, c0:c1], in_=ot[:, c0:c1])

            PRO = min(6, NPK) if nb > 1 else 0
            wqd = {n: load_w(n) for n in range(min(3, NPK))}

            def w_of(n):
                if n not in wqd:
                    wqd[n] = load_w(n)
                return wqd[n]

            for n in range(PRO):  # prologue: b0-only columns
                ots[n] = op.tile([P, tmax], f32, name=f"o_{n}", tag="o")
                if n + 3 < NPK:
                    w_of(n + 3)
                cell(n, 0, w_of(n))
            for n in range(NPK):
                if n >= PRO:
                    ots[n] = op.tile([P, tmax], f32, name=f"o_{n}", tag="o")
                    if n + 3 < NPK:
                        w_of(n + 3)
                w = wqd.pop(n)
                for b in range(0 if n >= PRO else 1, nb):
                    cell(n, b, w)
    nc.compile()
    return nc


def _plan(counts):
    """Allocate 8 cores to experts proportionally (largest remainder),
    then split each expert's token range into per-core contiguous spans.
    Returns (spans, t_max): spans[c] = (expert, start, length)."""
    total = int(counts.sum())
    ne = len(counts)
    active = [e for e in range(ne) if counts[e] > 0]
    quota = {e: counts[e] * NCORES / total for e in active}
    alloc = {e: max(1, int(quota[e])) for e in active}
    while sum(alloc.values()) > NCORES:  # too many mins; shrink largest
        shrinkable = [e for e in active if alloc[e] > 1]
        e = max(shrinkable, key=lambda e: alloc[e] - quota[e])
        alloc[e] -= 1
    rema = sorted(active, key=lambda e: quota[e] - alloc[e], reverse=True)
    i = 0
    while sum(alloc.values()) < NCORES:
        alloc[rema[i % len(rema)]] += 1
        i += 1
    spans = []
    starts = np.concatenate([[0], np.cumsum(counts)])
    for e in active:
        k = alloc[e]
        base, extra = divmod(int(counts[e]), k)
        off = int(starts[e])
        for j in range(k):
            ln = base + (1 if j < extra else 0)
            spans.append((e, off, ln))
            off += ln
    t_max = max(ln for _, _, ln in spans)
    t_max = max(P, t_max + (t_max & 1))
    return spans, t_max


def _runner(t_max: int):
    """Compiled 8-core executor for the mt-tile module, cached so repeat
    kernel() calls skip jax retracing. Mirrors bass2jax.run_bass_via_pjrt's
    multi-core path (concat per-core inputs on axis 0 + shard_map)."""
    import jax
    from jax.sharding import Mesh, PartitionSpec
    from jax.experimental.shard_map import shard_map
    from concourse import bass2jax, mybir as mb

    nc = _build(t_max)
    bass2jax.install_neuronx_cc_hook()

    part_name = nc.partition_id_tensor.name if nc.partition_id_tensor else None
    in_names, out_names, out_avals = [], [], []
    for alloc in nc.m.functions[0].allocations:
        if not isinstance(alloc, mb.MemoryLocationSet):
            continue
        name = alloc.memorylocations[0].name
        if alloc.kind == "ExternalInput":
            if name != part_name:
                in_names.append(name)
        elif alloc.kind == "ExternalOutput":
            out_names.append(name)
            out_avals.append(
                jax.core.ShapedArray(tuple(alloc.tensor_shape),
                                     mb.dt.np(alloc.dtype)))
    n_params = len(in_names)
    n_outs = len(out_names)
    bind_names = in_names + out_names + ([part_name] if part_name else [])

    def _body(*args):
        operands = list(args)
        if part_name:
            operands.append(bass2jax.partition_id_tensor())
        outs = bass2jax._bass_exec_p.bind(
            *operands,
            out_avals=tuple(out_avals),
            in_names=tuple(bind_names),
            out_names=tuple(out_names),
            lowering_input_output_aliases=(),
            sim_require_finite=True,
            sim_require_nnan=True,
            nc=nc,
        )
        return tuple(outs)

    devices = jax.devices()[:NCORES]
    mesh = Mesh(np.asarray(devices), ("core",))
    sharded = jax.jit(
        shard_map(_body, mesh=mesh,
                  in_specs=(PartitionSpec("core"),) * (n_params + n_outs),
                  out_specs=(PartitionSpec("core"),) * n_outs,
                  check_rep=False),
        donate_argnums=tuple(range(n_params, n_params + n_outs)),
        keep_unused=True,
    )

    def run(in_maps):
        concat_in = [
            np.concatenate([m[name] for m in in_maps], axis=0)
            for name in in_names
        ]
        zeros = [np.zeros((NCORES * a.shape[0], *a.shape[1:]), a.dtype)
                 for a in out_avals]
        outs = sharded(*concat_in, *zeros)
        return [
            {name: np.asarray(outs[i]).reshape(NCORES, *out_avals[i].shape)[c]
             for i, name in enumerate(out_names)}
            for c in range(NCORES)
        ]

    return run


def _pack_w(We: np.ndarray) -> np.ndarray:
    """wp[n*128+kk, ko*128+j] = We[n*128+j, ko*128+kk], cast bf16."""
    return np.ascontiguousarray(
        We.reshape(NPK, P, KO, P).transpose(0, 3, 2, 1)
        .reshape(QKV_OUT, HIDDEN).astype(BF16))


def _pack_x(xs: np.ndarray, blks) -> np.ndarray:
    """xp[p, (b, ko, c)] = xs[start_b + c, ko*128 + p] for block-major
    contiguous per-block DMA delivery. xs: [t_max, HIDDEN] bf16."""
    t_max = xs.shape[0]
    parts = []
    c0 = 0
    for bw in blks:
        chunk = xs[c0:c0 + bw].reshape(bw, KO, P).transpose(2, 1, 0)
        parts.append(chunk.reshape(P, KO * bw))
        c0 += bw
    return np.ascontiguousarray(np.concatenate(parts, axis=1))


def make_inputs(x, W, spans, t_max):
    blks = _blocks(t_max)
    wps = {}
    in_maps = []
    for e, off, ln in spans:
        if e not in wps:
            wps[e] = _pack_w(np.asarray(W[e]))
        xs = np.zeros((t_max, HIDDEN), dtype=BF16)
        xs[:ln] = x[off:off + ln].astype(BF16)
        in_maps.append({"xp": _pack_x(xs, blks), "wp": wps[e]})
    return in_maps


def kernel(x, W, modality_mapping):
    x = np.ascontiguousarray(np.asarray(x, dtype=np.float32))
    W = np.asarray(W, dtype=np.float32)
    mm = np.asarray(modality_mapping)

    perm = None
    if np.any(np.diff(mm) < 0):  # insurance: tokens not pre-sorted
        perm = np.argsort(mm, kind="stable")
        x = x[perm]
        mm = mm[perm]

    T = x.shape[0]
    E = W.shape[0]
    counts = np.bincount(mm.astype(np.int64), minlength=E)
    spans, t_max = _plan(counts)

    if t_max not in _cache:
        _cache[t_max] = _runner(t_max)
    run = _cache[t_max]

    results = run(make_inputs(x, W, spans, t_max))

    out = np.empty((T, QKV_OUT), dtype=np.float32)
    for c, (e, off, ln) in enumerate(spans):
        out[off:off + ln] = results[c]["outT"][:, :ln].T
    if perm is not None:
        inv = np.empty_like(perm)
        inv[perm] = np.arange(T)
        out = out[inv]
    return out
